# revision 41
# baseline (speedup 1.0000x reference)
"""Causal self-attention (B=2, T=2048, C=1024, H=16) on 8 TRN2 NeuronCores.

Sharding: batch x head-group. Core c handles batch b = c//4 and heads
[4g, 4g+4) with g = c%4.

v3 structure (all bf16 on-chip, fp32 PSUM accumulate):
  - host pre-transposes x -> xT [C, T] and converts inputs to bf16
  - warmup matmuls on memset tiles fill the initial DMA wait and get the
    PE clock to full rate before real work lands
  - emission order B-pair0 -> C -> D-pair0 -> B-pair1 -> D-pair1 lets the
    scheduler drop pair1 projection matmuls into D-pair0's exp stalls
  - PSUM->SBUF copies go to ACT during B/C (ACT idle there), DVE during D
  - softmax denominators bounce through DRAM into [128, 8] for the
    reciprocal (DVE reciprocal cost is free-dim-driven)
  - ones column of V via gpsimd memset
Host sums the 4 partial y's per batch (row-parallel unshard).
"""
import os
import sys

sys.path.insert(0, "/opt/trn_rl_repo")

import numpy as np
import ml_dtypes

try:
    import antenv.axon_hooks  # noqa: F401
except ImportError:
    import types
    import antenv
    _m = types.ModuleType("antenv.axon_hooks")
    _m._HOOK = None
    _m.set_axon_ntff_profile_hook = lambda h: setattr(_m, "_HOOK", h)
    _m.get_axon_ntff_profile_hook = lambda: _m._HOOK
    sys.modules["antenv.axon_hooks"] = _m
    antenv.axon_hooks = _m

import concourse.bass as bass
import concourse.mybir as mybir
import concourse.tile as tile
from concourse import bacc
from concourse import bass_utils
from concourse.masks import make_identity

P = 128
B, T, C = 2, 2048, 1024
H, HD = 16, 64
N_CORES = 8
HEADS_PER_CORE = H // 4          # 4
PAIRS = HEADS_PER_CORE // 2      # 2
TT = T // P                      # 16 t-tiles
CT = C // P                      # 8 c-tiles
QC = 512                         # q-chunk size
NQC = T // QC                    # 4 q-chunks
SCALE = 1.0 / np.sqrt(HD)
N_WARMUP = int(os.environ.get("ATTN_WARMUP", "24"))
MASK_NEG = -1.0e9

F32 = mybir.dt.float32
BF16 = mybir.dt.bfloat16
NP_BF16 = ml_dtypes.bfloat16

_NC_CACHE = None
LAST_RESULTS = None


def _build():
    nc = bacc.Bacc("TRN2", target_bir_lowering=False, debug=False,
                   enable_asserts=True, num_devices=1)
    xt = nc.dram_tensor("xt", [C, T], BF16, kind="ExternalInput").ap()
    wqk = nc.dram_tensor("wqk", [C, 512], BF16, kind="ExternalInput").ap()
    wv = nc.dram_tensor("wv", [C, 256], BF16, kind="ExternalInput").ap()
    sel = nc.dram_tensor("sel", [2, P], BF16, kind="ExternalInput").ap()
    wout = nc.dram_tensor("wout", [256, C], BF16, kind="ExternalInput").ap()
    y = nc.dram_tensor("y", [T, C], BF16, kind="ExternalOutput").ap()

    with tile.TileContext(nc) as tc:
        _emit(nc, tc, xt, wqk, wv, wout, sel, y)
    nc.compile()
    return nc


def _emit(nc, tc, xt, wqk, wv, wout, sel, y):
    import contextlib
    with contextlib.ExitStack() as ctx:
        ep = ctx.enter_context
        consts = ep(tc.tile_pool(name="consts", bufs=1))
        qkt_pool = ep(tc.tile_pool(name="qkt", bufs=1))
        v_pool = ep(tc.tile_pool(name="v", bufs=1))
        w_pool = ep(tc.tile_pool(name="w", bufs=1))
        xt_pool = ep(tc.tile_pool(name="xt", bufs=1))
        ot_pool = ep(tc.tile_pool(name="ot", bufs=1))
        est_pool = ep(tc.tile_pool(name="est", bufs=6))
        sb_misc = ep(tc.tile_pool(name="misc", bufs=3))
        ysb_pool = ep(tc.tile_pool(name="ysb", bufs=3))
        dram_tmp = ep(tc.tile_pool(name="dram_tmp", bufs=2, space="DRAM"))
        early = contextlib.ExitStack()
        ps_qk = early.enter_context(tc.tile_pool(name="ps_qk", bufs=2,
                                                 space="PSUM"))
        ps_v = early.enter_context(tc.tile_pool(name="ps_v", bufs=2,
                                                space="PSUM"))

        # ---------------- constants (no DMA deps) ----------------
        # tri mask [128, 128]: 1.0 where q >= k (keep), 0 above the diagonal
        maskm = consts.tile([P, P], F32)
        nc.gpsimd.memset(maskm[:], 0.0)
        nc.gpsimd.affine_select(
            out=maskm[:], in_=maskm[:], compare_op=mybir.AluOpType.is_gt,
            fill=1.0, base=0, pattern=[[-1, P]], channel_multiplier=1)
        mask_c = consts.tile([P, P], BF16)
        nc.vector.tensor_copy(mask_c[:], maskm[:])
        maskr = mask_c[:]

        warm_a = consts.tile([P, P], BF16)
        nc.gpsimd.memset(warm_a[:], 0.125)
        warm_b = consts.tile([P, 512], BF16)
        nc.gpsimd.memset(warm_b[:], 0.125)

        sel_sb = consts.tile([2, P], BF16)
        nc.sync.dma_start(sel_sb[:], sel)

        # ---------------- DMAs, dependency-order ----------------
        wqk_sb = w_pool.tile([P, CT, 512], BF16)
        nc.sync.dma_start(wqk_sb[:], wqk.rearrange("(co p) n -> p co n", p=P))

        # single xT tile, one DMA job per 512-wide t-chunk (fewer DIRECT2D
        # programming slots on the sync sequencer)
        xT_t = xt_pool.tile([P, CT, T], BF16, name="xT")
        xT = [xT_t[:, ci, :] for ci in range(CT)]

        def load_xt(tch):
            tsl = slice(tch * 512, (tch + 1) * 512)
            nc.sync.dma_start(
                xT_t[:, :, tsl],
                xt.rearrange("(co p) t -> p co t", p=P)[:, :, tsl])

        load_xt(0)
        wv_sb = w_pool.tile([P, CT, 256], BF16)
        nc.sync.dma_start(wv_sb[:], wv.rearrange("(co p) n -> p co n", p=P))
        for tch in range(1, T // 512):
            load_xt(tch)
        wout_sb = w_pool.tile([P, 2, C], BF16)
        nc.sync.dma_start(wout_sb[:], wout.rearrange("(pr p) n -> p pr n", p=P))

        # ---------------- PE warmup during the DMA ramp ----------------
        pw = ps_qk.tile([P, 512], F32, tag="qk", name="pw")
        for i in range(N_WARMUP):
            nc.tensor.matmul(pw[:], warm_a[:], warm_b[:],
                             start=(i == 0), stop=(i == N_WARMUP - 1))
        # keep the warmup live past DCE with a tiny DRAM write
        wsink = sb_misc.tile([1, 8], F32, tag="wsink", name="wsink")
        nc.vector.tensor_copy(wsink[:], pw[0:1, 0:8])
        dwarm = dram_tmp.tile([1, 8], F32, name="dwarm")
        nc.sync.dma_start(dwarm[:], wsink[:])

        qkt = [qkt_pool.tile([P, T], BF16, tag=f"qkt{ch}", name=f"qkt{ch}")
               for ch in range(4)]
        # V padded to 128 columns per head: a 128-wide bf16 stationary
        # operand gets FWL (fast weight load); PSUM rows 65..127 are unused
        v_sb = [v_pool.tile([P, HEADS_PER_CORE, P], BF16, tag=f"v{ti}",
                            name=f"v{ti}") for ti in range(TT)]

        # ---------------- phase B: Q^T/K^T projections ----------------
        # chunk layout: 0 = pair0 Q (headA|headB), 1 = pair0 K, 2/3 = pair1
        def emit_b(ch, pool, tag, copy_eng):
            for tch in range(T // 512):
                pq = pool.tile([P, 512], F32, tag=tag)
                for ci in range(CT):
                    nc.tensor.matmul(
                        pq[:], wqk_sb[:, ci, ch * P:(ch + 1) * P],
                        xT[ci][:, tch * 512:(tch + 1) * 512],
                        start=(ci == 0), stop=(ci == CT - 1))
                copy_eng(qkt[ch][:, tch * 512:(tch + 1) * 512], pq[:])

        # ACT is idle outside attention; it takes the pair-0 PSUM copies
        emit_b(0, ps_qk, "qk", nc.scalar.copy)
        emit_b(1, ps_qk, "qk", nc.scalar.copy)

        # ---------------- phase C: V (+ones col) ----------------
        for ti in range(TT):
            pv = ps_v.tile([P, 256], F32, tag="v")
            for ci in range(CT):
                nc.tensor.matmul(
                    pv[:], xT[ci][:, ti * P:(ti + 1) * P], wv_sb[:, ci],
                    start=(ci == 0), stop=(ci == CT - 1))
            nc.scalar.copy(
                v_sb[ti][:, :, 0:HD],
                pv[:].rearrange("p (h d) -> p h d", h=HEADS_PER_CORE))
            nc.gpsimd.memset(v_sb[ti][:, :, HD], 1.0)
            nc.gpsimd.memset(v_sb[ti][:, :, HD + 1:], 0.0)

        # close B/C PSUM pools so phase D pools get their banks
        early.close()
        ps_s = ep(tc.tile_pool(name="ps_s", bufs=2, space="PSUM"))
        ps_o = ep(tc.tile_pool(name="ps_o", bufs=1, space="PSUM"))
        ps_by = ep(tc.tile_pool(name="ps_by", bufs=2, space="PSUM"))

        # ---------------- phase D: attention ----------------
        ot = [ot_pool.tile([P, T], BF16, tag=f"ot{p}", name=f"ot{p}")
              for p in range(PAIRS)]

        def emit_out_proj(ti):
            ysb = ysb_pool.tile([P, C], BF16, tag="y", name="ysb")
            for cc in range(C // 512):
                py = ps_by.tile([P, 512], F32, tag="by", name="py")
                for pp in range(PAIRS):
                    nc.tensor.matmul(
                        py[:], ot[pp][:, ti * P:(ti + 1) * P],
                        wout_sb[:, pp, cc * 512:(cc + 1) * 512],
                        start=(pp == 0), stop=(pp == PAIRS - 1))
                # split PSUM->bf16 casts across DVE and ACT
                ysl = slice(cc * 512, (cc + 1) * 512)
                if cc == 0:
                    nc.vector.tensor_copy(ysb[:, ysl], py[:])
                else:
                    nc.scalar.copy(ysb[:, ysl], py[:])
            nc.sync.dma_start(y[ti * P:(ti + 1) * P, :], ysb[:])

        pending_norm = []

        def emit_norm(p, qc, po):
            # stash raw O into ot; SBUF->SBUF DMA spreads the denom row
            # into a [32, 32] layout (DVE reciprocal cost is free-dim
            # driven), reciprocal there, DMA back to [2, QC], one sel
            # matmul, multiply ot in place straight from PSUM.
            last = (p == PAIRS - 1 and qc == NQC - 1)
            qsl = slice(qc * QC, (qc + 1) * QC)
            stage = sb_misc.tile([HD + 1, 2 * QC], F32, tag="stage",
                                 name="stage")
            if last:
                # final group: interleave the denom row per 128-q quarter
                # ([h0 | h1] contiguous per quarter) and run four
                # independent chains so the closing out_projs overlap the
                # DMA bounce latency
                for half in range(2):
                    nc.vector.tensor_copy(
                        ot[p][half * HD:(half + 1) * HD, qsl],
                        po[half][0:HD, :])
                    for i in range(4):
                        nc.vector.tensor_copy(
                            stage[HD:HD + 1,
                                  i * 2 * P + half * P:
                                  i * 2 * P + (half + 1) * P],
                            po[half][HD:HD + 1, i * P:(i + 1) * P])
                den_q = []
                for i in range(4):
                    dq = sb_misc.tile([32, 8], F32, tag=f"denq{i}",
                                      name=f"denq{i}")
                    nc.sync.dma_start(
                        dq[:], stage[HD:HD + 1, i * 2 * P:(i + 1) * 2 * P])
                    den_q.append(dq)
            else:
                for half in range(2):
                    nc.vector.tensor_copy(
                        ot[p][half * HD:(half + 1) * HD, qsl],
                        po[half][0:HD, :])
                    nc.vector.tensor_copy(
                        stage[HD:HD + 1, half * QC:(half + 1) * QC],
                        po[half][HD:HD + 1, :])
                den32 = sb_misc.tile([32, 32], F32, tag="den", name="den32")
                nc.sync.dma_start(den32[:], stage[HD:HD + 1, :])

            def finish():
                if last:
                    for i in range(4):
                        rq = sb_misc.tile([32, 8], BF16, tag=f"recipq{i}",
                                          name=f"recipq{i}")
                        with nc.allow_low_precision(reason="bf16 recip"):
                            nc.vector.reciprocal(rq[:], den_q[i][:])
                        rq2 = sb_misc.tile([2, P], BF16, tag=f"recipq2{i}",
                                           name=f"recipq2{i}")
                        nc.sync.dma_start(rq2[:], rq[:])
                        pb = ps_by.tile([P, P], F32, tag="by", name="pbq")
                        nc.tensor.matmul(pb[:], sel_sb[:], rq2[:],
                                         start=True, stop=True)
                        isl = slice(qc * QC + i * P, qc * QC + (i + 1) * P)
                        nc.vector.tensor_tensor(
                            ot[p][:, isl], ot[p][:, isl],
                            pb[:], mybir.AluOpType.mult)
                        emit_out_proj(4 * qc + i)
                    return
                recip32 = sb_misc.tile([32, 32], BF16, tag="recip",
                                       name="recip32")
                with nc.allow_low_precision(reason="bf16 recip"):
                    nc.vector.reciprocal(recip32[:], den32[:])
                recip2 = sb_misc.tile([2, QC], BF16, tag="recip2",
                                      name="recip2")
                nc.sync.dma_start(recip2[:], recip32[:])
                pb = ps_by.tile([P, QC], F32, tag="by", name="pb")
                nc.tensor.matmul(pb[:], sel_sb[:], recip2[:],
                                 start=True, stop=True)
                # per-t-tile multiply so out_proj(ti) can start while the
                # later tiles' normalization is still running
                for i in range(4):
                    isl = slice(qc * QC + i * P, qc * QC + (i + 1) * P)
                    nc.vector.tensor_tensor(
                        ot[p][:, isl], ot[p][:, isl],
                        pb[:, i * P:(i + 1) * P], mybir.AluOpType.mult)
                    if p == PAIRS - 1:
                        emit_out_proj(4 * qc + i)
            pending_norm.append(finish)

        def drain_norm():
            while pending_norm:
                pending_norm.pop(0)()

        def emit_d(p):
            qt_t, kt_t = qkt[2 * p], qkt[2 * p + 1]
            for qc in range(NQC):
                kmax = 4 * (qc + 1)
                po = [ps_o.tile([P, QC], F32, tag=f"o{h}", name=f"po{h}")
                      for h in range(2)]
                for kt in range(kmax):
                    r = kt - (kmax - 4)
                    sp = max(r, 0) * P     # valid q-span starts here
                    ksl = slice(kt * P, (kt + 1) * P)
                    qsub = slice(qc * QC + sp, (qc + 1) * QC)
                    ps = ps_s.tile([P, 2 * QC], F32, tag="s")
                    for half in range(2):
                        off = half * QC
                        nc.tensor.matmul(ps[:, off + sp:off + QC],
                                         kt_t[half * HD:(half + 1) * HD, ksl],
                                         qt_t[half * HD:(half + 1) * HD, qsub],
                                         start=True, stop=True,
                                         tile_position=(half * HD, 0))
                    est = est_pool.tile([P, 2 * QC], BF16, tag="est")
                    if sp <= P:
                        nc.scalar.activation(est[:, sp:], ps[:, sp:],
                                             mybir.ActivationFunctionType.Exp,
                                             scale=SCALE)
                    else:
                        for half in range(2):
                            off = half * QC
                            nc.scalar.activation(
                                est[:, off + sp:off + QC],
                                ps[:, off + sp:off + QC],
                                mybir.ActivationFunctionType.Exp,
                                scale=SCALE)
                    if r >= 0:
                        for half in range(2):
                            off = half * QC
                            nc.vector.tensor_tensor(
                                est[:, off + r * P: off + (r + 1) * P],
                                est[:, off + r * P: off + (r + 1) * P],
                                maskr[:], mybir.AluOpType.mult)
                    for half in range(2):
                        nc.tensor.matmul(
                            po[half][:, sp:],
                            v_sb[kt][:, 2 * p + half],
                            est[:, half * QC + sp:(half + 1) * QC],
                            start=(kt == 0), stop=(kt == kmax - 1))
                emit_norm(p, qc, po)
                while len(pending_norm) >= 3:
                    pending_norm.pop(0)()

        emit_d(0)
        # DVE for pair-1 copies: ACT (exp) paces phase D, keep it clear
        emit_b(2, ps_by, "by", nc.vector.tensor_copy)
        emit_b(3, ps_by, "by", nc.vector.tensor_copy)
        emit_d(1)
        drain_norm()


def _get_nc():
    global _NC_CACHE
    if _NC_CACHE is None:
        _NC_CACHE = _build()
    return _NC_CACHE


def kernel(x, w_qkv, w_out):
    global LAST_RESULTS
    x = np.asarray(x, dtype=np.float32)
    w_qkv = np.asarray(w_qkv, dtype=np.float32)
    w_out = np.asarray(w_out, dtype=np.float32)

    wq, wk, wv = w_qkv[:, 0:C], w_qkv[:, C:2 * C], w_qkv[:, 2 * C:3 * C]

    xt_all = [np.ascontiguousarray(x[b].T).astype(NP_BF16) for b in range(B)]

    in_maps = []
    for c in range(N_CORES):
        b, g = c // 4, c % 4
        heads = [4 * g + i for i in range(HEADS_PER_CORE)]
        cols = lambda w, h: w[:, h * HD:(h + 1) * HD]
        wqk_c = np.concatenate([
            cols(wq, heads[0]), cols(wq, heads[1]),
            cols(wk, heads[0]), cols(wk, heads[1]),
            cols(wq, heads[2]), cols(wq, heads[3]),
            cols(wk, heads[2]), cols(wk, heads[3]),
        ], axis=1)
        wv_c = wv[:, heads[0] * HD:(heads[-1] + 1) * HD]
        wout_c = w_out[heads[0] * HD:(heads[-1] + 1) * HD, :]
        sel_np = np.zeros((2, 128), dtype=NP_BF16)
        sel_np[0, 0:64] = 1.0
        sel_np[1, 64:128] = 1.0
        in_maps.append({
            "xt": xt_all[b],
            "sel": sel_np,
            "wqk": np.ascontiguousarray(wqk_c).astype(NP_BF16),
            "wv": np.ascontiguousarray(wv_c).astype(NP_BF16),
            "wout": np.ascontiguousarray(wout_c).astype(NP_BF16),
        })

    nc = _get_nc()
    res = bass_utils.run_bass_kernel_spmd(
        nc, in_maps, core_ids=list(range(N_CORES)),
        trace=bool(os.environ.get("ATTN_TRACE")))
    LAST_RESULTS = res

    out = np.zeros((B, T, C), dtype=np.float64)
    for c in range(N_CORES):
        out[c // 4] += res.results[c]["y"].astype(np.float64)
    return out.astype(np.float32)


# revision 42
# speedup vs baseline: 1.1573x; 1.1573x over previous
"""Causal self-attention (B=2, T=2048, C=1024, H=16) on 8 TRN2 NeuronCores.

Sharding: batch x head-group. Core c handles batch b = c//4 and heads
[4g, 4g+4) with g = c%4.

v3 structure (all bf16 on-chip, fp32 PSUM accumulate):
  - host pre-transposes x -> xT [C, T] and converts inputs to bf16
  - warmup matmuls on memset tiles fill the initial DMA wait and get the
    PE clock to full rate before real work lands
  - emission order B-pair0 -> C -> D-pair0 -> B-pair1 -> D-pair1 lets the
    scheduler drop pair1 projection matmuls into D-pair0's exp stalls
  - PSUM->SBUF copies go to ACT during B/C (ACT idle there), DVE during D
  - softmax denominators bounce through DRAM into [128, 8] for the
    reciprocal (DVE reciprocal cost is free-dim-driven)
  - ones column of V via gpsimd memset
Host sums the 4 partial y's per batch (row-parallel unshard).
"""
import os
import sys

sys.path.insert(0, "/opt/trn_rl_repo")

import numpy as np
import ml_dtypes

try:
    import antenv.axon_hooks  # noqa: F401
except ImportError:
    import types
    import antenv
    _m = types.ModuleType("antenv.axon_hooks")
    _m._HOOK = None
    _m.set_axon_ntff_profile_hook = lambda h: setattr(_m, "_HOOK", h)
    _m.get_axon_ntff_profile_hook = lambda: _m._HOOK
    sys.modules["antenv.axon_hooks"] = _m
    antenv.axon_hooks = _m

import concourse.bass as bass
import concourse.mybir as mybir
import concourse.tile as tile
from concourse import bacc
from concourse import bass_utils
from concourse.masks import make_identity

P = 128
B, T, C = 2, 2048, 1024
H, HD = 16, 64
N_CORES = 8
HEADS_PER_CORE = H // 4          # 4
PAIRS = HEADS_PER_CORE // 2      # 2
TT = T // P                      # 16 t-tiles
CT = C // P                      # 8 c-tiles
QC = 512                         # q-chunk size
NQC = T // QC                    # 4 q-chunks
SCALE = 1.0 / np.sqrt(HD)
N_WARMUP = int(os.environ.get("ATTN_WARMUP", "24"))
MASK_NEG = -1.0e9

F32 = mybir.dt.float32
BF16 = mybir.dt.bfloat16
NP_BF16 = ml_dtypes.bfloat16

_NC_CACHE = None
LAST_RESULTS = None


def _build():
    nc = bacc.Bacc("TRN2", target_bir_lowering=False, debug=False,
                   enable_asserts=True, num_devices=1)
    xt = nc.dram_tensor("xt", [C, T], BF16, kind="ExternalInput").ap()
    wqk = nc.dram_tensor("wqk", [C, 512], BF16, kind="ExternalInput").ap()
    wv = nc.dram_tensor("wv", [C, 256], BF16, kind="ExternalInput").ap()
    sel = nc.dram_tensor("sel", [2, P], BF16, kind="ExternalInput").ap()
    wout = nc.dram_tensor("wout", [256, C], BF16, kind="ExternalInput").ap()
    y = nc.dram_tensor("y", [T, C], BF16, kind="ExternalOutput").ap()

    with tile.TileContext(nc) as tc:
        _emit(nc, tc, xt, wqk, wv, wout, sel, y)
    nc.compile()
    return nc


def _emit(nc, tc, xt, wqk, wv, wout, sel, y):
    import contextlib
    with contextlib.ExitStack() as ctx:
        ep = ctx.enter_context
        consts = ep(tc.tile_pool(name="consts", bufs=1))
        qkt_pool = ep(tc.tile_pool(name="qkt", bufs=1))
        v_pool = ep(tc.tile_pool(name="v", bufs=1))
        w_pool = ep(tc.tile_pool(name="w", bufs=1))
        xt_pool = ep(tc.tile_pool(name="xt", bufs=1))
        ot_pool = ep(tc.tile_pool(name="ot", bufs=1))
        est_pool = ep(tc.tile_pool(name="est", bufs=6))
        sb_misc = ep(tc.tile_pool(name="misc", bufs=3))
        ysb_pool = ep(tc.tile_pool(name="ysb", bufs=3))
        dram_tmp = ep(tc.tile_pool(name="dram_tmp", bufs=2, space="DRAM"))
        early = contextlib.ExitStack()
        ps_qk = early.enter_context(tc.tile_pool(name="ps_qk", bufs=2,
                                                 space="PSUM"))
        ps_v = early.enter_context(tc.tile_pool(name="ps_v", bufs=2,
                                                space="PSUM"))

        # ---------------- constants (no DMA deps) ----------------
        # tri mask [128, 128]: 1.0 where q >= k (keep), 0 above the diagonal
        maskm = consts.tile([P, P], F32)
        nc.gpsimd.memset(maskm[:], 0.0)
        nc.gpsimd.affine_select(
            out=maskm[:], in_=maskm[:], compare_op=mybir.AluOpType.is_gt,
            fill=1.0, base=0, pattern=[[-1, P]], channel_multiplier=1)
        mask_c = consts.tile([P, P], BF16)
        nc.vector.tensor_copy(mask_c[:], maskm[:])
        maskr = mask_c[:]

        warm_a = consts.tile([P, P], BF16)
        nc.gpsimd.memset(warm_a[:], 0.125)
        warm_b = consts.tile([P, 512], BF16)
        nc.gpsimd.memset(warm_b[:], 0.125)

        sel_sb = consts.tile([2, P], BF16)
        nc.sync.dma_start(sel_sb[:], sel)

        # ---------------- DMAs, dependency-order ----------------
        wqk_sb = w_pool.tile([P, CT, 512], BF16)
        nc.sync.dma_start(wqk_sb[:], wqk.rearrange("(co p) n -> p co n", p=P))

        # single xT tile, one DMA job per 512-wide t-chunk (fewer DIRECT2D
        # programming slots on the sync sequencer)
        xT_t = xt_pool.tile([P, CT, T], BF16, name="xT")
        xT = [xT_t[:, ci, :] for ci in range(CT)]

        def load_xt(tch):
            tsl = slice(tch * 512, (tch + 1) * 512)
            nc.sync.dma_start(
                xT_t[:, :, tsl],
                xt.rearrange("(co p) t -> p co t", p=P)[:, :, tsl])

        load_xt(0)
        wv_sb = w_pool.tile([P, CT, 256], BF16)
        nc.sync.dma_start(wv_sb[:], wv.rearrange("(co p) n -> p co n", p=P))
        for tch in range(1, T // 512):
            load_xt(tch)
        wout_sb = w_pool.tile([P, 2, C], BF16)
        nc.sync.dma_start(wout_sb[:], wout.rearrange("(pr p) n -> p pr n", p=P))

        # ---------------- PE warmup during the DMA ramp ----------------
        pw = ps_qk.tile([P, 512], F32, tag="qk", name="pw")
        for i in range(N_WARMUP):
            nc.tensor.matmul(pw[:], warm_a[:], warm_b[:],
                             start=(i == 0), stop=(i == N_WARMUP - 1))
        # keep the warmup live past DCE with a tiny DRAM write
        wsink = sb_misc.tile([1, 8], F32, tag="wsink", name="wsink")
        nc.vector.tensor_copy(wsink[:], pw[0:1, 0:8])
        dwarm = dram_tmp.tile([1, 8], F32, name="dwarm")
        nc.sync.dma_start(dwarm[:], wsink[:])

        qkt = [qkt_pool.tile([P, T], BF16, tag=f"qkt{ch}", name=f"qkt{ch}")
               for ch in range(4)]
        # V padded to 128 columns per head: a 128-wide bf16 stationary
        # operand gets FWL (fast weight load); PSUM rows 65..127 are unused
        v_sb = [v_pool.tile([P, HEADS_PER_CORE, P], BF16, tag=f"v{ti}",
                            name=f"v{ti}") for ti in range(TT)]

        # ---------------- phase B: Q^T/K^T projections ----------------
        # chunk layout: 0 = pair0 Q (headA|headB), 1 = pair0 K, 2/3 = pair1
        def emit_b(ch, pool, tag, copy_eng):
            for tch in range(T // 512):
                pq = pool.tile([P, 512], F32, tag=tag)
                for ci in range(CT):
                    nc.tensor.matmul(
                        pq[:], wqk_sb[:, ci, ch * P:(ch + 1) * P],
                        xT[ci][:, tch * 512:(tch + 1) * 512],
                        start=(ci == 0), stop=(ci == CT - 1))
                copy_eng(qkt[ch][:, tch * 512:(tch + 1) * 512], pq[:])

        # ACT is idle outside attention; it takes the pair-0 PSUM copies
        emit_b(0, ps_qk, "qk", nc.scalar.copy)
        emit_b(1, ps_qk, "qk", nc.scalar.copy)

        # ---------------- phase C: V (+ones col) ----------------
        for ti in range(TT):
            pv = ps_v.tile([P, 256], F32, tag="v")
            for ci in range(CT):
                nc.tensor.matmul(
                    pv[:], xT[ci][:, ti * P:(ti + 1) * P], wv_sb[:, ci],
                    start=(ci == 0), stop=(ci == CT - 1))
            nc.scalar.copy(
                v_sb[ti][:, :, 0:HD],
                pv[:].rearrange("p (h d) -> p h d", h=HEADS_PER_CORE))
            nc.gpsimd.memset(v_sb[ti][:, :, HD], 1.0)
            nc.gpsimd.memset(v_sb[ti][:, :, HD + 1:], 0.0)

        # close B/C PSUM pools so phase D pools get their banks
        early.close()
        ps_s = ep(tc.tile_pool(name="ps_s", bufs=2, space="PSUM"))
        ps_o = ep(tc.tile_pool(name="ps_o", bufs=1, space="PSUM"))
        ps_by = ep(tc.tile_pool(name="ps_by", bufs=2, space="PSUM"))

        # ---------------- phase D: attention ----------------
        ot = [ot_pool.tile([P, T], BF16, tag=f"ot{p}", name=f"ot{p}")
              for p in range(PAIRS)]

        def emit_out_proj(ti):
            ysb = ysb_pool.tile([P, C], BF16, tag="y", name="ysb")
            for cc in range(C // 512):
                py = ps_by.tile([P, 512], F32, tag="by", name="py")
                for pp in range(PAIRS):
                    nc.tensor.matmul(
                        py[:], ot[pp][:, ti * P:(ti + 1) * P],
                        wout_sb[:, pp, cc * 512:(cc + 1) * 512],
                        start=(pp == 0), stop=(pp == PAIRS - 1))
                # split PSUM->bf16 casts across DVE and ACT
                ysl = slice(cc * 512, (cc + 1) * 512)
                if cc == 0:
                    nc.vector.tensor_copy(ysb[:, ysl], py[:])
                else:
                    nc.scalar.copy(ysb[:, ysl], py[:])
            nc.sync.dma_start(y[ti * P:(ti + 1) * P, :], ysb[:])

        pending_norm = []

        def emit_norm(p, qc, po):
            # stash raw O into ot; SBUF->SBUF DMA spreads the denom row
            # into a [32, 32] layout (DVE reciprocal cost is free-dim
            # driven), reciprocal there, DMA back to [2, QC], one sel
            # matmul, multiply ot in place straight from PSUM.
            last = (p == PAIRS - 1 and qc == NQC - 1)
            qsl = slice(qc * QC, (qc + 1) * QC)
            stage = sb_misc.tile([HD + 1, 2 * QC], F32, tag="stage",
                                 name="stage")
            if last:
                # final group: interleave the denom row per 128-q quarter
                # ([h0 | h1] contiguous per quarter) and run four
                # independent chains so the closing out_projs overlap the
                # DMA bounce latency
                for half in range(2):
                    nc.vector.tensor_copy(
                        ot[p][half * HD:(half + 1) * HD, qsl],
                        po[half][0:HD, :])
                    for i in range(4):
                        nc.vector.tensor_copy(
                            stage[HD:HD + 1,
                                  i * 2 * P + half * P:
                                  i * 2 * P + (half + 1) * P],
                            po[half][HD:HD + 1, i * P:(i + 1) * P])
                den_q = []
                for i in range(4):
                    dq = sb_misc.tile([32, 8], F32, tag=f"denq{i}",
                                      name=f"denq{i}")
                    nc.sync.dma_start(
                        dq[:], stage[HD:HD + 1, i * 2 * P:(i + 1) * 2 * P])
                    den_q.append(dq)
            else:
                for half in range(2):
                    nc.vector.tensor_copy(
                        ot[p][half * HD:(half + 1) * HD, qsl],
                        po[half][0:HD, :])
                    nc.vector.tensor_copy(
                        stage[HD:HD + 1, half * QC:(half + 1) * QC],
                        po[half][HD:HD + 1, :])
                den32 = sb_misc.tile([32, 32], F32, tag="den", name="den32")
                nc.sync.dma_start(den32[:], stage[HD:HD + 1, :])

            def finish():
                if last:
                    for i in range(4):
                        rq = sb_misc.tile([32, 8], BF16, tag=f"recipq{i}",
                                          name=f"recipq{i}")
                        with nc.allow_low_precision(reason="bf16 recip"):
                            nc.vector.reciprocal(rq[:], den_q[i][:])
                        rq2 = sb_misc.tile([2, P], BF16, tag=f"recipq2{i}",
                                           name=f"recipq2{i}")
                        nc.sync.dma_start(rq2[:], rq[:])
                        pb = ps_by.tile([P, P], F32, tag="by", name="pbq")
                        nc.tensor.matmul(pb[:], sel_sb[:], rq2[:],
                                         start=True, stop=True)
                        isl = slice(qc * QC + i * P, qc * QC + (i + 1) * P)
                        nc.vector.tensor_tensor(
                            ot[p][:, isl], ot[p][:, isl],
                            pb[:], mybir.AluOpType.mult)
                        emit_out_proj(4 * qc + i)
                    return
                recip32 = sb_misc.tile([32, 32], BF16, tag="recip",
                                       name="recip32")
                with nc.allow_low_precision(reason="bf16 recip"):
                    nc.vector.reciprocal(recip32[:], den32[:])
                recip2 = sb_misc.tile([2, QC], BF16, tag="recip2",
                                      name="recip2")
                nc.sync.dma_start(recip2[:], recip32[:])
                pb = ps_by.tile([P, QC], F32, tag="by", name="pb")
                nc.tensor.matmul(pb[:], sel_sb[:], recip2[:],
                                 start=True, stop=True)
                # per-t-tile multiply so out_proj(ti) can start while the
                # later tiles' normalization is still running
                for i in range(4):
                    isl = slice(qc * QC + i * P, qc * QC + (i + 1) * P)
                    nc.vector.tensor_tensor(
                        ot[p][:, isl], ot[p][:, isl],
                        pb[:, i * P:(i + 1) * P], mybir.AluOpType.mult)
                    if p == PAIRS - 1:
                        emit_out_proj(4 * qc + i)
            pending_norm.append(finish)

        def drain_norm():
            while pending_norm:
                pending_norm.pop(0)()

        def emit_d(p):
            qt_t, kt_t = qkt[2 * p], qkt[2 * p + 1]
            for qc in range(NQC):
                kmax = 4 * (qc + 1)
                po = [ps_o.tile([P, QC], F32, tag=f"o{h}", name=f"po{h}")
                      for h in range(2)]
                for kt in range(kmax):
                    r = kt - (kmax - 4)
                    sp = max(r, 0) * P     # valid q-span starts here
                    ksl = slice(kt * P, (kt + 1) * P)
                    qsub = slice(qc * QC + sp, (qc + 1) * QC)
                    ps = ps_s.tile([P, 2 * QC], F32, tag="s")
                    for half in range(2):
                        off = half * QC
                        nc.tensor.matmul(ps[:, off + sp:off + QC],
                                         kt_t[half * HD:(half + 1) * HD, ksl],
                                         qt_t[half * HD:(half + 1) * HD, qsub],
                                         start=True, stop=True,
                                         tile_position=(half * HD, 0))
                    est = est_pool.tile([P, 2 * QC], BF16, tag="est")
                    # one exp instruction per block: the [QC:QC+sp] middle
                    # is never-read garbage, but one wide instruction beats
                    # two narrow ones (~290ns fixed cost each)
                    nc.scalar.activation(est[:, sp:], ps[:, sp:],
                                         mybir.ActivationFunctionType.Exp,
                                         scale=SCALE)
                    if r >= 0:
                        for half in range(2):
                            off = half * QC
                            nc.vector.tensor_tensor(
                                est[:, off + r * P: off + (r + 1) * P],
                                est[:, off + r * P: off + (r + 1) * P],
                                maskr[:], mybir.AluOpType.mult)
                    for half in range(2):
                        nc.tensor.matmul(
                            po[half][:, sp:],
                            v_sb[kt][:, 2 * p + half],
                            est[:, half * QC + sp:(half + 1) * QC],
                            start=(kt == 0), stop=(kt == kmax - 1))
                emit_norm(p, qc, po)
                while len(pending_norm) >= 3:
                    pending_norm.pop(0)()

        emit_d(0)
        # DVE for pair-1 copies: ACT (exp) paces phase D, keep it clear
        emit_b(2, ps_by, "by", nc.vector.tensor_copy)
        emit_b(3, ps_by, "by", nc.vector.tensor_copy)
        emit_d(1)
        drain_norm()


def _get_nc():
    global _NC_CACHE
    if _NC_CACHE is None:
        _NC_CACHE = _build()
    return _NC_CACHE


def kernel(x, w_qkv, w_out):
    global LAST_RESULTS
    x = np.asarray(x, dtype=np.float32)
    w_qkv = np.asarray(w_qkv, dtype=np.float32)
    w_out = np.asarray(w_out, dtype=np.float32)

    wq, wk, wv = w_qkv[:, 0:C], w_qkv[:, C:2 * C], w_qkv[:, 2 * C:3 * C]

    xt_all = [np.ascontiguousarray(x[b].T).astype(NP_BF16) for b in range(B)]

    in_maps = []
    for c in range(N_CORES):
        b, g = c // 4, c % 4
        heads = [4 * g + i for i in range(HEADS_PER_CORE)]
        cols = lambda w, h: w[:, h * HD:(h + 1) * HD]
        wqk_c = np.concatenate([
            cols(wq, heads[0]), cols(wq, heads[1]),
            cols(wk, heads[0]), cols(wk, heads[1]),
            cols(wq, heads[2]), cols(wq, heads[3]),
            cols(wk, heads[2]), cols(wk, heads[3]),
        ], axis=1)
        wv_c = wv[:, heads[0] * HD:(heads[-1] + 1) * HD]
        wout_c = w_out[heads[0] * HD:(heads[-1] + 1) * HD, :]
        sel_np = np.zeros((2, 128), dtype=NP_BF16)
        sel_np[0, 0:64] = 1.0
        sel_np[1, 64:128] = 1.0
        in_maps.append({
            "xt": xt_all[b],
            "sel": sel_np,
            "wqk": np.ascontiguousarray(wqk_c).astype(NP_BF16),
            "wv": np.ascontiguousarray(wv_c).astype(NP_BF16),
            "wout": np.ascontiguousarray(wout_c).astype(NP_BF16),
        })

    nc = _get_nc()
    res = bass_utils.run_bass_kernel_spmd(
        nc, in_maps, core_ids=list(range(N_CORES)),
        trace=bool(os.environ.get("ATTN_TRACE")))
    LAST_RESULTS = res

    out = np.zeros((B, T, C), dtype=np.float64)
    for c in range(N_CORES):
        out[c // 4] += res.results[c]["y"].astype(np.float64)
    return out.astype(np.float32)


# revision 45
# speedup vs baseline: 1.1602x; 1.0025x over previous
"""Causal self-attention (B=2, T=2048, C=1024, H=16) on 8 TRN2 NeuronCores.

Sharding: batch x head-group. Core c handles batch b = c//4 and heads
[4g, 4g+4) with g = c%4.

v3 structure (all bf16 on-chip, fp32 PSUM accumulate):
  - host pre-transposes x -> xT [C, T] and converts inputs to bf16
  - warmup matmuls on memset tiles fill the initial DMA wait and get the
    PE clock to full rate before real work lands
  - emission order B-pair0 -> C -> D-pair0 -> B-pair1 -> D-pair1 lets the
    scheduler drop pair1 projection matmuls into D-pair0's exp stalls
  - PSUM->SBUF copies go to ACT during B/C (ACT idle there), DVE during D
  - softmax denominators bounce through DRAM into [128, 8] for the
    reciprocal (DVE reciprocal cost is free-dim-driven)
  - ones column of V via gpsimd memset
Host sums the 4 partial y's per batch (row-parallel unshard).
"""
import os
import sys

sys.path.insert(0, "/opt/trn_rl_repo")

import numpy as np
import ml_dtypes

try:
    import antenv.axon_hooks  # noqa: F401
except ImportError:
    import types
    import antenv
    _m = types.ModuleType("antenv.axon_hooks")
    _m._HOOK = None
    _m.set_axon_ntff_profile_hook = lambda h: setattr(_m, "_HOOK", h)
    _m.get_axon_ntff_profile_hook = lambda: _m._HOOK
    sys.modules["antenv.axon_hooks"] = _m
    antenv.axon_hooks = _m

import concourse.bass as bass
import concourse.mybir as mybir
import concourse.tile as tile
from concourse import bacc
from concourse import bass_utils
from concourse.masks import make_identity

P = 128
B, T, C = 2, 2048, 1024
H, HD = 16, 64
N_CORES = 8
HEADS_PER_CORE = H // 4          # 4
PAIRS = HEADS_PER_CORE // 2      # 2
TT = T // P                      # 16 t-tiles
CT = C // P                      # 8 c-tiles
QC = 512                         # q-chunk size
NQC = T // QC                    # 4 q-chunks
SCALE = 1.0 / np.sqrt(HD)
N_WARMUP = int(os.environ.get("ATTN_WARMUP", "28"))
MASK_NEG = -1.0e9

F32 = mybir.dt.float32
BF16 = mybir.dt.bfloat16
NP_BF16 = ml_dtypes.bfloat16

_NC_CACHE = None
LAST_RESULTS = None


def _build():
    nc = bacc.Bacc("TRN2", target_bir_lowering=False, debug=False,
                   enable_asserts=True, num_devices=1)
    xt = nc.dram_tensor("xt", [C, T], BF16, kind="ExternalInput").ap()
    wqk = nc.dram_tensor("wqk", [C, 512], BF16, kind="ExternalInput").ap()
    wv = nc.dram_tensor("wv", [C, 256], BF16, kind="ExternalInput").ap()
    sel = nc.dram_tensor("sel", [2, P], BF16, kind="ExternalInput").ap()
    wout = nc.dram_tensor("wout", [256, C], BF16, kind="ExternalInput").ap()
    y = nc.dram_tensor("y", [T, C], BF16, kind="ExternalOutput").ap()

    with tile.TileContext(nc) as tc:
        _emit(nc, tc, xt, wqk, wv, wout, sel, y)
    nc.compile()
    return nc


def _emit(nc, tc, xt, wqk, wv, wout, sel, y):
    import contextlib
    with contextlib.ExitStack() as ctx:
        ep = ctx.enter_context
        consts = ep(tc.tile_pool(name="consts", bufs=1))
        qkt_pool = ep(tc.tile_pool(name="qkt", bufs=1))
        v_pool = ep(tc.tile_pool(name="v", bufs=1))
        w_pool = ep(tc.tile_pool(name="w", bufs=1))
        xt_pool = ep(tc.tile_pool(name="xt", bufs=1))
        ot_pool = ep(tc.tile_pool(name="ot", bufs=1))
        est_pool = ep(tc.tile_pool(name="est", bufs=6))
        sb_misc = ep(tc.tile_pool(name="misc", bufs=3))
        ysb_pool = ep(tc.tile_pool(name="ysb", bufs=3))
        dram_tmp = ep(tc.tile_pool(name="dram_tmp", bufs=2, space="DRAM"))
        early = contextlib.ExitStack()
        ps_qk = early.enter_context(tc.tile_pool(name="ps_qk", bufs=2,
                                                 space="PSUM"))
        ps_v = early.enter_context(tc.tile_pool(name="ps_v", bufs=2,
                                                space="PSUM"))

        # ---------------- constants (no DMA deps) ----------------
        # tri mask [128, 128]: 1.0 where q >= k (keep), 0 above the diagonal
        maskm = consts.tile([P, P], F32)
        nc.gpsimd.memset(maskm[:], 0.0)
        nc.gpsimd.affine_select(
            out=maskm[:], in_=maskm[:], compare_op=mybir.AluOpType.is_gt,
            fill=1.0, base=0, pattern=[[-1, P]], channel_multiplier=1)
        mask_c = consts.tile([P, P], BF16)
        nc.vector.tensor_copy(mask_c[:], maskm[:])
        maskr = mask_c[:]

        warm_a = consts.tile([P, P], BF16)
        nc.gpsimd.memset(warm_a[:], 0.125)
        warm_b = consts.tile([P, 512], BF16)
        nc.gpsimd.memset(warm_b[:], 0.125)

        sel_sb = consts.tile([2, P], BF16)
        nc.sync.dma_start(sel_sb[:], sel)

        # ---------------- DMAs, dependency-order ----------------
        wqk_sb = w_pool.tile([P, CT, 512], BF16)
        nc.sync.dma_start(wqk_sb[:], wqk.rearrange("(co p) n -> p co n", p=P))

        # single xT tile, one DMA job per 512-wide t-chunk (fewer DIRECT2D
        # programming slots on the sync sequencer)
        xT_t = xt_pool.tile([P, CT, T], BF16, name="xT")
        xT = [xT_t[:, ci, :] for ci in range(CT)]

        def load_xt(tch):
            tsl = slice(tch * 512, (tch + 1) * 512)
            nc.sync.dma_start(
                xT_t[:, :, tsl],
                xt.rearrange("(co p) t -> p co t", p=P)[:, :, tsl])

        load_xt(0)
        wv_sb = w_pool.tile([P, CT, 256], BF16)
        nc.sync.dma_start(wv_sb[:], wv.rearrange("(co p) n -> p co n", p=P))
        for tch in range(1, T // 512):
            load_xt(tch)
        wout_sb = w_pool.tile([P, 2, C], BF16)
        nc.sync.dma_start(wout_sb[:], wout.rearrange("(pr p) n -> p pr n", p=P))

        # ---------------- PE warmup during the DMA ramp ----------------
        pw = ps_qk.tile([P, 512], F32, tag="qk", name="pw")
        for i in range(N_WARMUP):
            nc.tensor.matmul(pw[:], warm_a[:], warm_b[:],
                             start=(i == 0), stop=(i == N_WARMUP - 1))
        # keep the warmup live past DCE with a tiny DRAM write
        wsink = sb_misc.tile([1, 8], F32, tag="wsink", name="wsink")
        nc.vector.tensor_copy(wsink[:], pw[0:1, 0:8])
        dwarm = dram_tmp.tile([1, 8], F32, name="dwarm")
        nc.sync.dma_start(dwarm[:], wsink[:])

        qkt = [qkt_pool.tile([P, T], BF16, tag=f"qkt{ch}", name=f"qkt{ch}")
               for ch in range(4)]
        # V padded to 128 columns per head: a 128-wide bf16 stationary
        # operand gets FWL (fast weight load); PSUM rows 65..127 are unused
        v_sb = [v_pool.tile([P, HEADS_PER_CORE, P], BF16, tag=f"v{ti}",
                            name=f"v{ti}") for ti in range(TT)]

        # ---------------- phase B: Q^T/K^T projections ----------------
        # chunk layout: 0 = pair0 Q (headA|headB), 1 = pair0 K, 2/3 = pair1
        def emit_b(ch, pool, tag, copy_eng):
            for tch in range(T // 512):
                pq = pool.tile([P, 512], F32, tag=tag)
                for ci in range(CT):
                    nc.tensor.matmul(
                        pq[:], wqk_sb[:, ci, ch * P:(ch + 1) * P],
                        xT[ci][:, tch * 512:(tch + 1) * 512],
                        start=(ci == 0), stop=(ci == CT - 1))
                copy_eng(qkt[ch][:, tch * 512:(tch + 1) * 512], pq[:])

        # ACT is idle outside attention; it takes the pair-0 PSUM copies
        emit_b(0, ps_qk, "qk", nc.scalar.copy)
        emit_b(1, ps_qk, "qk", nc.scalar.copy)

        # ---------------- phase C: V (+ones col) ----------------
        for ti in range(TT):
            pv = ps_v.tile([P, 256], F32, tag="v")
            for ci in range(CT):
                nc.tensor.matmul(
                    pv[:], xT[ci][:, ti * P:(ti + 1) * P], wv_sb[:, ci],
                    start=(ci == 0), stop=(ci == CT - 1))
            nc.scalar.copy(
                v_sb[ti][:, :, 0:HD],
                pv[:].rearrange("p (h d) -> p h d", h=HEADS_PER_CORE))
            nc.gpsimd.memset(v_sb[ti][:, :, HD], 1.0)
            nc.gpsimd.memset(v_sb[ti][:, :, HD + 1:], 0.0)

        # close B/C PSUM pools so phase D pools get their banks
        early.close()
        ps_s = ep(tc.tile_pool(name="ps_s", bufs=2, space="PSUM"))
        ps_o = ep(tc.tile_pool(name="ps_o", bufs=1, space="PSUM"))
        ps_by = ep(tc.tile_pool(name="ps_by", bufs=2, space="PSUM"))

        # ---------------- phase D: attention ----------------
        ot = [ot_pool.tile([P, T], BF16, tag=f"ot{p}", name=f"ot{p}")
              for p in range(PAIRS)]

        def emit_out_proj(ti):
            ysb = ysb_pool.tile([P, C], BF16, tag="y", name="ysb")
            for cc in range(C // 512):
                py = ps_by.tile([P, 512], F32, tag="by", name="py")
                for pp in range(PAIRS):
                    nc.tensor.matmul(
                        py[:], ot[pp][:, ti * P:(ti + 1) * P],
                        wout_sb[:, pp, cc * 512:(cc + 1) * 512],
                        start=(pp == 0), stop=(pp == PAIRS - 1))
                # split PSUM->bf16 casts across DVE and ACT
                ysl = slice(cc * 512, (cc + 1) * 512)
                if cc == 0:
                    nc.vector.tensor_copy(ysb[:, ysl], py[:])
                else:
                    nc.scalar.copy(ysb[:, ysl], py[:])
            nc.sync.dma_start(y[ti * P:(ti + 1) * P, :], ysb[:])

        pending_norm = []

        def emit_norm(p, qc, po):
            # stash raw O into ot; SBUF->SBUF DMA spreads the denom row
            # into a [32, 32] layout (DVE reciprocal cost is free-dim
            # driven), reciprocal there, DMA back to [2, QC], one sel
            # matmul, multiply ot in place straight from PSUM.
            last = (p == PAIRS - 1 and qc == NQC - 1)
            qsl = slice(qc * QC, (qc + 1) * QC)
            stage = sb_misc.tile([HD + 1, 2 * QC], F32, tag="stage",
                                 name="stage")
            if last:
                # final group: interleave the denom row per 128-q quarter
                # ([h0 | h1] contiguous per quarter); one SBUF->SBUF hop
                # per quarter straight into [2, 128] so the closing
                # out_projs start as early as possible
                for half in range(2):
                    for i in range(4):
                        nc.vector.tensor_copy(
                            stage[HD:HD + 1,
                                  i * 2 * P + half * P:
                                  i * 2 * P + (half + 1) * P],
                            po[half][HD:HD + 1, i * P:(i + 1) * P])
                den_q = []
                for i in range(4):
                    dq = sb_misc.tile([2, P], F32, tag=f"denq{i}",
                                      name=f"denq{i}")
                    nc.sync.dma_start(
                        dq[:], stage[HD:HD + 1, i * 2 * P:(i + 1) * 2 * P])
                    den_q.append(dq)
                for half in range(2):
                    nc.vector.tensor_copy(
                        ot[p][half * HD:(half + 1) * HD, qsl],
                        po[half][0:HD, :])
            else:
                for half in range(2):
                    nc.vector.tensor_copy(
                        stage[HD:HD + 1, half * QC:(half + 1) * QC],
                        po[half][HD:HD + 1, :])
                den32 = sb_misc.tile([32, 32], F32, tag="den", name="den32")
                nc.sync.dma_start(den32[:], stage[HD:HD + 1, :])
                for half in range(2):
                    nc.vector.tensor_copy(
                        ot[p][half * HD:(half + 1) * HD, qsl],
                        po[half][0:HD, :])

            def finish():
                if last:
                    for i in range(4):
                        rq2 = sb_misc.tile([2, P], BF16, tag=f"recipq2{i}",
                                           name=f"recipq2{i}")
                        with nc.allow_low_precision(reason="bf16 recip"):
                            nc.vector.reciprocal(rq2[:], den_q[i][:])
                        pb = ps_by.tile([P, P], F32, tag="by", name="pbq")
                        nc.tensor.matmul(pb[:], sel_sb[:], rq2[:],
                                         start=True, stop=True)
                        isl = slice(qc * QC + i * P, qc * QC + (i + 1) * P)
                        nc.vector.tensor_tensor(
                            ot[p][:, isl], ot[p][:, isl],
                            pb[:], mybir.AluOpType.mult)
                        emit_out_proj(4 * qc + i)
                    return
                recip32 = sb_misc.tile([32, 32], BF16, tag="recip",
                                       name="recip32")
                with nc.allow_low_precision(reason="bf16 recip"):
                    nc.vector.reciprocal(recip32[:], den32[:])
                recip2 = sb_misc.tile([2, QC], BF16, tag="recip2",
                                      name="recip2")
                nc.sync.dma_start(recip2[:], recip32[:])
                pb = ps_by.tile([P, QC], F32, tag="by", name="pb")
                nc.tensor.matmul(pb[:], sel_sb[:], recip2[:],
                                 start=True, stop=True)
                # per-t-tile multiply so out_proj(ti) can start while the
                # later tiles' normalization is still running
                for i in range(4):
                    isl = slice(qc * QC + i * P, qc * QC + (i + 1) * P)
                    nc.vector.tensor_tensor(
                        ot[p][:, isl], ot[p][:, isl],
                        pb[:, i * P:(i + 1) * P], mybir.AluOpType.mult)
                    if p == PAIRS - 1:
                        emit_out_proj(4 * qc + i)
            pending_norm.append(finish)

        def drain_norm():
            while pending_norm:
                pending_norm.pop(0)()

        def emit_d(p):
            qt_t, kt_t = qkt[2 * p], qkt[2 * p + 1]
            for qc in range(NQC):
                kmax = 4 * (qc + 1)
                po = [ps_o.tile([P, QC], F32, tag=f"o{h}", name=f"po{h}")
                      for h in range(2)]
                for kt in range(kmax):
                    r = kt - (kmax - 4)
                    sp = max(r, 0) * P     # valid q-span starts here
                    ksl = slice(kt * P, (kt + 1) * P)
                    qsub = slice(qc * QC + sp, (qc + 1) * QC)
                    ps = ps_s.tile([P, 2 * QC], F32, tag="s")
                    for half in range(2):
                        off = half * QC
                        nc.tensor.matmul(ps[:, off + sp:off + QC],
                                         kt_t[half * HD:(half + 1) * HD, ksl],
                                         qt_t[half * HD:(half + 1) * HD, qsub],
                                         start=True, stop=True,
                                         tile_position=(half * HD, 0))
                    est = est_pool.tile([P, 2 * QC], BF16, tag="est")
                    # one exp instruction per block: the [QC:QC+sp] middle
                    # is never-read garbage, but one wide instruction beats
                    # two narrow ones (~290ns fixed cost each)
                    nc.scalar.activation(est[:, sp:], ps[:, sp:],
                                         mybir.ActivationFunctionType.Exp,
                                         scale=SCALE)
                    if r >= 0:
                        for half in range(2):
                            off = half * QC
                            nc.vector.tensor_tensor(
                                est[:, off + r * P: off + (r + 1) * P],
                                est[:, off + r * P: off + (r + 1) * P],
                                maskr[:], mybir.AluOpType.mult)
                    for half in range(2):
                        nc.tensor.matmul(
                            po[half][:, sp:],
                            v_sb[kt][:, 2 * p + half],
                            est[:, half * QC + sp:(half + 1) * QC],
                            start=(kt == 0), stop=(kt == kmax - 1))
                emit_norm(p, qc, po)
                # deep pipeline in pair 0; drain tighter in pair 1 so only
                # the final group's chain is exposed at the end
                depth = 3 if p == 0 else 2
                while len(pending_norm) >= depth:
                    pending_norm.pop(0)()

        emit_d(0)
        # DVE for pair-1 copies: ACT (exp) paces phase D, keep it clear
        emit_b(2, ps_by, "by", nc.vector.tensor_copy)
        emit_b(3, ps_by, "by", nc.vector.tensor_copy)
        emit_d(1)
        drain_norm()


def _get_nc():
    global _NC_CACHE
    if _NC_CACHE is None:
        _NC_CACHE = _build()
    return _NC_CACHE


def kernel(x, w_qkv, w_out):
    global LAST_RESULTS
    x = np.asarray(x, dtype=np.float32)
    w_qkv = np.asarray(w_qkv, dtype=np.float32)
    w_out = np.asarray(w_out, dtype=np.float32)

    wq, wk, wv = w_qkv[:, 0:C], w_qkv[:, C:2 * C], w_qkv[:, 2 * C:3 * C]

    xt_all = [np.ascontiguousarray(x[b].T).astype(NP_BF16) for b in range(B)]

    in_maps = []
    for c in range(N_CORES):
        b, g = c // 4, c % 4
        heads = [4 * g + i for i in range(HEADS_PER_CORE)]
        cols = lambda w, h: w[:, h * HD:(h + 1) * HD]
        wqk_c = np.concatenate([
            cols(wq, heads[0]), cols(wq, heads[1]),
            cols(wk, heads[0]), cols(wk, heads[1]),
            cols(wq, heads[2]), cols(wq, heads[3]),
            cols(wk, heads[2]), cols(wk, heads[3]),
        ], axis=1)
        wv_c = wv[:, heads[0] * HD:(heads[-1] + 1) * HD]
        wout_c = w_out[heads[0] * HD:(heads[-1] + 1) * HD, :]
        sel_np = np.zeros((2, 128), dtype=NP_BF16)
        sel_np[0, 0:64] = 1.0
        sel_np[1, 64:128] = 1.0
        in_maps.append({
            "xt": xt_all[b],
            "sel": sel_np,
            "wqk": np.ascontiguousarray(wqk_c).astype(NP_BF16),
            "wv": np.ascontiguousarray(wv_c).astype(NP_BF16),
            "wout": np.ascontiguousarray(wout_c).astype(NP_BF16),
        })

    nc = _get_nc()
    res = bass_utils.run_bass_kernel_spmd(
        nc, in_maps, core_ids=list(range(N_CORES)),
        trace=bool(os.environ.get("ATTN_TRACE")))
    LAST_RESULTS = res

    out = np.zeros((B, T, C), dtype=np.float64)
    for c in range(N_CORES):
        out[c // 4] += res.results[c]["y"].astype(np.float64)
    return out.astype(np.float32)


# revision 47
# speedup vs baseline: 1.1627x; 1.0022x over previous
"""Causal self-attention (B=2, T=2048, C=1024, H=16) on 8 TRN2 NeuronCores.

Sharding: batch x head-group. Core c handles batch b = c//4 and heads
[4g, 4g+4) with g = c%4.

v3 structure (all bf16 on-chip, fp32 PSUM accumulate):
  - host pre-transposes x -> xT [C, T] and converts inputs to bf16
  - warmup matmuls on memset tiles fill the initial DMA wait and get the
    PE clock to full rate before real work lands
  - emission order B-pair0 -> C -> D-pair0 -> B-pair1 -> D-pair1 lets the
    scheduler drop pair1 projection matmuls into D-pair0's exp stalls
  - PSUM->SBUF copies go to ACT during B/C (ACT idle there), DVE during D
  - softmax denominators bounce through DRAM into [128, 8] for the
    reciprocal (DVE reciprocal cost is free-dim-driven)
  - ones column of V via gpsimd memset
Host sums the 4 partial y's per batch (row-parallel unshard).
"""
import os
import sys

sys.path.insert(0, "/opt/trn_rl_repo")

import numpy as np
import ml_dtypes

try:
    import antenv.axon_hooks  # noqa: F401
except ImportError:
    import types
    import antenv
    _m = types.ModuleType("antenv.axon_hooks")
    _m._HOOK = None
    _m.set_axon_ntff_profile_hook = lambda h: setattr(_m, "_HOOK", h)
    _m.get_axon_ntff_profile_hook = lambda: _m._HOOK
    sys.modules["antenv.axon_hooks"] = _m
    antenv.axon_hooks = _m

import concourse.bass as bass
import concourse.mybir as mybir
import concourse.tile as tile
from concourse import bacc
from concourse import bass_utils

P = 128
B, T, C = 2, 2048, 1024
H, HD = 16, 64
N_CORES = 8
HEADS_PER_CORE = H // 4          # 4
PAIRS = HEADS_PER_CORE // 2      # 2
TT = T // P                      # 16 t-tiles
CT = C // P                      # 8 c-tiles
QC = 512                         # q-chunk size
NQC = T // QC                    # 4 q-chunks
SCALE = 1.0 / np.sqrt(HD)
N_WARMUP = int(os.environ.get("ATTN_WARMUP", "28"))

F32 = mybir.dt.float32
BF16 = mybir.dt.bfloat16
NP_BF16 = ml_dtypes.bfloat16

_NC_CACHE = None
LAST_RESULTS = None


def _build():
    nc = bacc.Bacc("TRN2", target_bir_lowering=False, debug=False,
                   enable_asserts=True, num_devices=1)
    xt = nc.dram_tensor("xt", [C, T], BF16, kind="ExternalInput").ap()
    wqk = nc.dram_tensor("wqk", [C, 512], BF16, kind="ExternalInput").ap()
    wv = nc.dram_tensor("wv", [C, 256], BF16, kind="ExternalInput").ap()
    sel = nc.dram_tensor("sel", [2, P], BF16, kind="ExternalInput").ap()
    wout = nc.dram_tensor("wout", [256, C], BF16, kind="ExternalInput").ap()
    y = nc.dram_tensor("y", [T, C], BF16, kind="ExternalOutput").ap()

    with tile.TileContext(nc) as tc:
        _emit(nc, tc, xt, wqk, wv, wout, sel, y)
    nc.compile()
    return nc


def _emit(nc, tc, xt, wqk, wv, wout, sel, y):
    import contextlib
    with contextlib.ExitStack() as ctx:
        ep = ctx.enter_context
        consts = ep(tc.tile_pool(name="consts", bufs=1))
        qkt_pool = ep(tc.tile_pool(name="qkt", bufs=1))
        v_pool = ep(tc.tile_pool(name="v", bufs=1))
        w_pool = ep(tc.tile_pool(name="w", bufs=1))
        xt_pool = ep(tc.tile_pool(name="xt", bufs=1))
        ot_pool = ep(tc.tile_pool(name="ot", bufs=1))
        est_pool = ep(tc.tile_pool(name="est", bufs=6))
        sb_misc = ep(tc.tile_pool(name="misc", bufs=3))
        ysb_pool = ep(tc.tile_pool(name="ysb", bufs=3))
        dram_tmp = ep(tc.tile_pool(name="dram_tmp", bufs=2, space="DRAM"))
        early = contextlib.ExitStack()
        ps_qk = early.enter_context(tc.tile_pool(name="ps_qk", bufs=2,
                                                 space="PSUM"))
        ps_v = early.enter_context(tc.tile_pool(name="ps_v", bufs=2,
                                                space="PSUM"))

        # ---------------- constants (no DMA deps) ----------------
        # tri mask [128, 128]: 1.0 where q >= k (keep), 0 above the diagonal
        maskm = consts.tile([P, P], F32)
        nc.gpsimd.memset(maskm[:], 0.0)
        nc.gpsimd.affine_select(
            out=maskm[:], in_=maskm[:], compare_op=mybir.AluOpType.is_gt,
            fill=1.0, base=0, pattern=[[-1, P]], channel_multiplier=1)
        mask_c = consts.tile([P, P], BF16)
        nc.vector.tensor_copy(mask_c[:], maskm[:])
        maskr = mask_c[:]

        warm_a = consts.tile([P, P], BF16)
        nc.gpsimd.memset(warm_a[:], 0.125)
        warm_b = consts.tile([P, 512], BF16)
        nc.gpsimd.memset(warm_b[:], 0.125)

        sel_sb = consts.tile([2, P], BF16)
        nc.sync.dma_start(sel_sb[:], sel)

        # ---------------- DMAs, dependency-order ----------------
        wqk_sb = w_pool.tile([P, CT, 512], BF16)
        nc.sync.dma_start(wqk_sb[:], wqk.rearrange("(co p) n -> p co n", p=P))

        # single xT tile, one DMA job per 512-wide t-chunk (fewer DIRECT2D
        # programming slots on the sync sequencer)
        xT_t = xt_pool.tile([P, CT, T], BF16, name="xT")
        xT = [xT_t[:, ci, :] for ci in range(CT)]

        def load_xt(tch):
            tsl = slice(tch * 512, (tch + 1) * 512)
            nc.sync.dma_start(
                xT_t[:, :, tsl],
                xt.rearrange("(co p) t -> p co t", p=P)[:, :, tsl])

        load_xt(0)
        wv_sb = w_pool.tile([P, CT, 256], BF16)
        nc.sync.dma_start(wv_sb[:], wv.rearrange("(co p) n -> p co n", p=P))
        for tch in range(1, T // 512):
            load_xt(tch)
        wout_sb = w_pool.tile([P, 2, C], BF16)
        nc.sync.dma_start(wout_sb[:], wout.rearrange("(pr p) n -> p pr n", p=P))

        # ---------------- PE warmup during the DMA ramp ----------------
        pw = ps_qk.tile([P, 512], F32, tag="qk", name="pw")
        for i in range(N_WARMUP):
            nc.tensor.matmul(pw[:], warm_a[:], warm_b[:],
                             start=(i == 0), stop=(i == N_WARMUP - 1))
        # keep the warmup live past DCE with a tiny DRAM write
        wsink = sb_misc.tile([1, 8], F32, tag="wsink", name="wsink")
        nc.vector.tensor_copy(wsink[:], pw[0:1, 0:8])
        dwarm = dram_tmp.tile([1, 8], F32, name="dwarm")
        nc.sync.dma_start(dwarm[:], wsink[:])

        qkt = [qkt_pool.tile([P, T], BF16, tag=f"qkt{ch}", name=f"qkt{ch}")
               for ch in range(4)]
        # V padded to 128 columns per head: a 128-wide bf16 stationary
        # operand gets FWL (fast weight load); PSUM rows 65..127 are unused
        v_sb = [v_pool.tile([P, HEADS_PER_CORE, P], BF16, tag=f"v{ti}",
                            name=f"v{ti}") for ti in range(TT)]

        # ---------------- phase B: Q^T/K^T projections ----------------
        # chunk layout: 0 = pair0 Q (headA|headB), 1 = pair0 K, 2/3 = pair1
        def emit_b(ch, pool, tag, copy_eng):
            for tch in range(T // 512):
                pq = pool.tile([P, 512], F32, tag=tag)
                for ci in range(CT):
                    nc.tensor.matmul(
                        pq[:], wqk_sb[:, ci, ch * P:(ch + 1) * P],
                        xT[ci][:, tch * 512:(tch + 1) * 512],
                        start=(ci == 0), stop=(ci == CT - 1))
                copy_eng(qkt[ch][:, tch * 512:(tch + 1) * 512], pq[:])

        # ACT is idle outside attention; it takes the pair-0 PSUM copies
        emit_b(0, ps_qk, "qk", nc.scalar.copy)
        emit_b(1, ps_qk, "qk", nc.scalar.copy)

        # ---------------- phase C: V (+ones col) ----------------
        for ti in range(TT):
            pv = ps_v.tile([P, 256], F32, tag="v")
            for ci in range(CT):
                nc.tensor.matmul(
                    pv[:], xT[ci][:, ti * P:(ti + 1) * P], wv_sb[:, ci],
                    start=(ci == 0), stop=(ci == CT - 1))
            nc.scalar.copy(
                v_sb[ti][:, :, 0:HD],
                pv[:].rearrange("p (h d) -> p h d", h=HEADS_PER_CORE))
            nc.gpsimd.memset(v_sb[ti][:, :, HD], 1.0)
            nc.gpsimd.memset(v_sb[ti][:, :, HD + 1:], 0.0)

        # close B/C PSUM pools so phase D pools get their banks
        early.close()
        ps_s = ep(tc.tile_pool(name="ps_s", bufs=2, space="PSUM"))
        ps_o = ep(tc.tile_pool(name="ps_o", bufs=1, space="PSUM"))
        ps_by = ep(tc.tile_pool(name="ps_by", bufs=2, space="PSUM"))

        # ---------------- phase D: attention ----------------
        ot = [ot_pool.tile([P, T], BF16, tag=f"ot{p}", name=f"ot{p}")
              for p in range(PAIRS)]

        def emit_out_proj(ti):
            ysb = ysb_pool.tile([P, C], BF16, tag="y", name="ysb")
            for cc in range(C // 512):
                py = ps_by.tile([P, 512], F32, tag="by", name="py")
                for pp in range(PAIRS):
                    nc.tensor.matmul(
                        py[:], ot[pp][:, ti * P:(ti + 1) * P],
                        wout_sb[:, pp, cc * 512:(cc + 1) * 512],
                        start=(pp == 0), stop=(pp == PAIRS - 1))
                # split PSUM->bf16 casts across DVE and ACT
                ysl = slice(cc * 512, (cc + 1) * 512)
                if cc == 0:
                    nc.vector.tensor_copy(ysb[:, ysl], py[:])
                else:
                    nc.scalar.copy(ysb[:, ysl], py[:])
            nc.sync.dma_start(y[ti * P:(ti + 1) * P, :], ysb[:])

        pending_norm = []

        def emit_norm(p, qc, po):
            # stash raw O into ot; SBUF->SBUF DMA spreads the denom row
            # into a [32, 32] layout (DVE reciprocal cost is free-dim
            # driven), reciprocal there, DMA back to [2, QC], one sel
            # matmul, multiply ot in place straight from PSUM.
            last = (p == PAIRS - 1 and qc == NQC - 1)
            qsl = slice(qc * QC, (qc + 1) * QC)
            stage = sb_misc.tile([HD + 1, 2 * QC], F32, tag="stage",
                                 name="stage")
            if last:
                # final group: interleave the denom row per 128-q quarter
                # ([h0 | h1] contiguous per quarter); one SBUF->SBUF hop
                # per quarter straight into [2, 128] so the closing
                # out_projs start as early as possible
                for half in range(2):
                    for i in range(4):
                        nc.vector.tensor_copy(
                            stage[HD:HD + 1,
                                  i * 2 * P + half * P:
                                  i * 2 * P + (half + 1) * P],
                            po[half][HD:HD + 1, i * P:(i + 1) * P])
                den_q = []
                for i in range(4):
                    dq = sb_misc.tile([2, P], F32, tag=f"denq{i}",
                                      name=f"denq{i}")
                    nc.sync.dma_start(
                        dq[:], stage[HD:HD + 1, i * 2 * P:(i + 1) * 2 * P])
                    den_q.append(dq)
                for half in range(2):
                    nc.vector.tensor_copy(
                        ot[p][half * HD:(half + 1) * HD, qsl],
                        po[half][0:HD, :])
            else:
                for half in range(2):
                    nc.vector.tensor_copy(
                        stage[HD:HD + 1, half * QC:(half + 1) * QC],
                        po[half][HD:HD + 1, :])
                den32 = sb_misc.tile([32, 32], F32, tag="den", name="den32")
                nc.sync.dma_start(den32[:], stage[HD:HD + 1, :])
                for half in range(2):
                    nc.vector.tensor_copy(
                        ot[p][half * HD:(half + 1) * HD, qsl],
                        po[half][0:HD, :])

            def finish():
                if last:
                    for i in range(4):
                        rq2 = sb_misc.tile([2, P], BF16, tag=f"recipq2{i}",
                                           name=f"recipq2{i}")
                        with nc.allow_low_precision(reason="bf16 recip"):
                            nc.vector.reciprocal(rq2[:], den_q[i][:])
                        pb = ps_by.tile([P, P], F32, tag="by", name="pbq")
                        nc.tensor.matmul(pb[:], sel_sb[:], rq2[:],
                                         start=True, stop=True)
                        isl = slice(qc * QC + i * P, qc * QC + (i + 1) * P)
                        nc.vector.tensor_tensor(
                            ot[p][:, isl], ot[p][:, isl],
                            pb[:], mybir.AluOpType.mult)
                        emit_out_proj(4 * qc + i)
                    return
                recip32 = sb_misc.tile([32, 32], BF16, tag="recip",
                                       name="recip32")
                with nc.allow_low_precision(reason="bf16 recip"):
                    nc.vector.reciprocal(recip32[:], den32[:])
                recip2 = sb_misc.tile([2, QC], BF16, tag="recip2",
                                      name="recip2")
                nc.sync.dma_start(recip2[:], recip32[:])
                pb = ps_by.tile([P, QC], F32, tag="by", name="pb")
                nc.tensor.matmul(pb[:], sel_sb[:], recip2[:],
                                 start=True, stop=True)
                # per-t-tile multiply so out_proj(ti) can start while the
                # later tiles' normalization is still running
                for i in range(4):
                    isl = slice(qc * QC + i * P, qc * QC + (i + 1) * P)
                    nc.vector.tensor_tensor(
                        ot[p][:, isl], ot[p][:, isl],
                        pb[:, i * P:(i + 1) * P], mybir.AluOpType.mult)
                    if p == PAIRS - 1:
                        emit_out_proj(4 * qc + i)
            pending_norm.append(finish)

        def drain_norm():
            while pending_norm:
                pending_norm.pop(0)()

        def emit_d(p):
            qt_t, kt_t = qkt[2 * p], qkt[2 * p + 1]
            for qc in range(NQC):
                kmax = 4 * (qc + 1)
                po = [ps_o.tile([P, QC], F32, tag=f"o{h}", name=f"po{h}")
                      for h in range(2)]
                for kt in range(kmax):
                    r = kt - (kmax - 4)
                    sp = max(r, 0) * P     # valid q-span starts here
                    ksl = slice(kt * P, (kt + 1) * P)
                    qsub = slice(qc * QC + sp, (qc + 1) * QC)
                    ps = ps_s.tile([P, 2 * QC], F32, tag="s")
                    for half in range(2):
                        off = half * QC
                        nc.tensor.matmul(ps[:, off + sp:off + QC],
                                         kt_t[half * HD:(half + 1) * HD, ksl],
                                         qt_t[half * HD:(half + 1) * HD, qsub],
                                         start=True, stop=True,
                                         tile_position=(half * HD, 0))
                    est = est_pool.tile([P, 2 * QC], BF16, tag="est")
                    # one exp instruction per block: the [QC:QC+sp] middle
                    # is never-read garbage, but one wide instruction beats
                    # two narrow ones (~290ns fixed cost each)
                    nc.scalar.activation(est[:, sp:], ps[:, sp:],
                                         mybir.ActivationFunctionType.Exp,
                                         scale=SCALE)
                    if r >= 0:
                        for half in range(2):
                            off = half * QC
                            nc.vector.tensor_tensor(
                                est[:, off + r * P: off + (r + 1) * P],
                                est[:, off + r * P: off + (r + 1) * P],
                                maskr[:], mybir.AluOpType.mult)
                    for half in range(2):
                        nc.tensor.matmul(
                            po[half][:, sp:],
                            v_sb[kt][:, 2 * p + half],
                            est[:, half * QC + sp:(half + 1) * QC],
                            start=(kt == 0), stop=(kt == kmax - 1))
                emit_norm(p, qc, po)
                # deep pipeline in pair 0; drain tighter in pair 1 so only
                # the final group's chain is exposed at the end
                depth = 3 if p == 0 else 2
                while len(pending_norm) >= depth:
                    pending_norm.pop(0)()

        emit_d(0)
        # DVE for pair-1 copies: ACT (exp) paces phase D, keep it clear
        emit_b(2, ps_by, "by", nc.vector.tensor_copy)
        emit_b(3, ps_by, "by", nc.vector.tensor_copy)
        emit_d(1)
        drain_norm()


def _get_nc():
    global _NC_CACHE
    if _NC_CACHE is None:
        _NC_CACHE = _build()
    return _NC_CACHE


def kernel(x, w_qkv, w_out):
    global LAST_RESULTS
    x = np.asarray(x, dtype=np.float32)
    w_qkv = np.asarray(w_qkv, dtype=np.float32)
    w_out = np.asarray(w_out, dtype=np.float32)

    wq, wk, wv = w_qkv[:, 0:C], w_qkv[:, C:2 * C], w_qkv[:, 2 * C:3 * C]

    xt_all = [np.ascontiguousarray(x[b].T).astype(NP_BF16) for b in range(B)]

    in_maps = []
    for c in range(N_CORES):
        b, g = c // 4, c % 4
        heads = [4 * g + i for i in range(HEADS_PER_CORE)]
        cols = lambda w, h: w[:, h * HD:(h + 1) * HD]
        wqk_c = np.concatenate([
            cols(wq, heads[0]), cols(wq, heads[1]),
            cols(wk, heads[0]), cols(wk, heads[1]),
            cols(wq, heads[2]), cols(wq, heads[3]),
            cols(wk, heads[2]), cols(wk, heads[3]),
        ], axis=1)
        wv_c = wv[:, heads[0] * HD:(heads[-1] + 1) * HD]
        wout_c = w_out[heads[0] * HD:(heads[-1] + 1) * HD, :]
        sel_np = np.zeros((2, 128), dtype=NP_BF16)
        sel_np[0, 0:64] = 1.0
        sel_np[1, 64:128] = 1.0
        in_maps.append({
            "xt": xt_all[b],
            "sel": sel_np,
            "wqk": np.ascontiguousarray(wqk_c).astype(NP_BF16),
            "wv": np.ascontiguousarray(wv_c).astype(NP_BF16),
            "wout": np.ascontiguousarray(wout_c).astype(NP_BF16),
        })

    nc = _get_nc()
    res = bass_utils.run_bass_kernel_spmd(
        nc, in_maps, core_ids=list(range(N_CORES)),
        trace=bool(os.environ.get("ATTN_TRACE")))
    LAST_RESULTS = res

    out = np.zeros((B, T, C), dtype=np.float64)
    for c in range(N_CORES):
        out[c // 4] += res.results[c]["y"].astype(np.float64)
    return out.astype(np.float32)


# revision 50
# speedup vs baseline: 1.1731x; 1.0089x over previous
"""Causal self-attention (B=2, T=2048, C=1024, H=16) on 8 TRN2 NeuronCores.

Sharding: batch x head-group. Core c handles batch b = c//4 and heads
[4g, 4g+4) with g = c%4.

Structure (all bf16 on-chip, fp32 PSUM accumulate):
  - host pre-transposes x -> xT [C, T] and converts inputs to bf16, so
    the on-chip transpose phase disappears and DMA bytes halve
  - warmup matmuls on memset tiles fill the initial DMA ramp and get the
    PE clock to full rate before real work lands
  - emission order B-pair0 -> C -> D-pair0 -> B-pair1 -> D-pair1 lets the
    scheduler drop pair1 projection matmuls into D-pair0's exp stalls
  - few, large DMA jobs (the sync sequencer serializes DIRECT2D
    programming at ~0.6us per job)
  - PSUM->SBUF copies go to ACT during B/C (ACT idle there), DVE during D
    (exp on ACT paces phase D - keep it clear)
  - softmax denominators are spread via SBUF->SBUF DMA into a [32, 32]
    tile for the reciprocal (DVE reciprocal cost is free-dim-driven),
    then DMA'd back to [2, QC] rows for the sel-matmul broadcast
  - the final group's normalization runs as four per-128-q chains with a
    single DMA hop each, so the closing out_projs overlap the DMA latency
  - ones column of V via gpsimd memset; V padded to 128 columns so the
    AV stationary operand gets FWL
Host sums the 4 partial y's per batch (row-parallel unshard).
"""
import os
import sys

sys.path.insert(0, "/opt/trn_rl_repo")

import numpy as np
import ml_dtypes

try:
    import antenv.axon_hooks  # noqa: F401
except ImportError:
    import types
    import antenv
    _m = types.ModuleType("antenv.axon_hooks")
    _m._HOOK = None
    _m.set_axon_ntff_profile_hook = lambda h: setattr(_m, "_HOOK", h)
    _m.get_axon_ntff_profile_hook = lambda: _m._HOOK
    sys.modules["antenv.axon_hooks"] = _m
    antenv.axon_hooks = _m

import concourse.bass as bass
import concourse.mybir as mybir
import concourse.tile as tile
from concourse import bacc
from concourse import bass_utils

P = 128
B, T, C = 2, 2048, 1024
H, HD = 16, 64
N_CORES = 8
HEADS_PER_CORE = H // 4          # 4
PAIRS = HEADS_PER_CORE // 2      # 2
TT = T // P                      # 16 t-tiles
CT = C // P                      # 8 c-tiles
QC = 512                         # q-chunk size
NQC = T // QC                    # 4 q-chunks
SCALE = 1.0 / np.sqrt(HD)
N_WARMUP = int(os.environ.get("ATTN_WARMUP", "28"))

F32 = mybir.dt.float32
BF16 = mybir.dt.bfloat16
NP_BF16 = ml_dtypes.bfloat16

_NC_CACHE = None
LAST_RESULTS = None


def _build():
    nc = bacc.Bacc("TRN2", target_bir_lowering=False, debug=False,
                   enable_asserts=True, num_devices=1)
    xt = nc.dram_tensor("xt", [C, T], BF16, kind="ExternalInput").ap()
    wqk = nc.dram_tensor("wqk", [C, 512], BF16, kind="ExternalInput").ap()
    wv = nc.dram_tensor("wv", [C, 256], BF16, kind="ExternalInput").ap()
    sel = nc.dram_tensor("sel", [2, P], BF16, kind="ExternalInput").ap()
    wout = nc.dram_tensor("wout", [256, C], BF16, kind="ExternalInput").ap()
    y = nc.dram_tensor("y", [T, C], BF16, kind="ExternalOutput").ap()

    with tile.TileContext(nc) as tc:
        _emit(nc, tc, xt, wqk, wv, wout, sel, y)
    nc.compile()
    return nc


def _emit(nc, tc, xt, wqk, wv, wout, sel, y):
    import contextlib
    with contextlib.ExitStack() as ctx:
        ep = ctx.enter_context
        consts = ep(tc.tile_pool(name="consts", bufs=1))
        qkt_pool = ep(tc.tile_pool(name="qkt", bufs=1))
        v_pool = ep(tc.tile_pool(name="v", bufs=1))
        w_pool = ep(tc.tile_pool(name="w", bufs=1))
        xt_pool = ep(tc.tile_pool(name="xt", bufs=1))
        ot_pool = ep(tc.tile_pool(name="ot", bufs=1))
        est_pool = ep(tc.tile_pool(name="est", bufs=6))
        sb_misc = ep(tc.tile_pool(name="misc", bufs=3))
        ysb_pool = ep(tc.tile_pool(name="ysb", bufs=3))
        dram_tmp = ep(tc.tile_pool(name="dram_tmp", bufs=2, space="DRAM"))
        early = contextlib.ExitStack()
        ps_qk = early.enter_context(tc.tile_pool(name="ps_qk", bufs=2,
                                                 space="PSUM"))
        ps_v = early.enter_context(tc.tile_pool(name="ps_v", bufs=2,
                                                space="PSUM"))

        # ---------------- constants (no DMA deps) ----------------
        # tri mask [128, 128]: 1.0 where q >= k (keep), 0 above the diagonal
        maskm = consts.tile([P, P], F32)
        nc.gpsimd.memset(maskm[:], 0.0)
        nc.gpsimd.affine_select(
            out=maskm[:], in_=maskm[:], compare_op=mybir.AluOpType.is_gt,
            fill=1.0, base=0, pattern=[[-1, P]], channel_multiplier=1)
        mask_c = consts.tile([P, P], BF16)
        nc.vector.tensor_copy(mask_c[:], maskm[:])
        maskr = mask_c[:]

        warm_a = consts.tile([P, P], BF16)
        nc.gpsimd.memset(warm_a[:], 0.125)
        warm_b = consts.tile([P, 512], BF16)
        nc.gpsimd.memset(warm_b[:], 0.125)

        sel_sb = consts.tile([2, P], BF16)
        nc.sync.dma_start(sel_sb[:], sel)

        # ---------------- DMAs, dependency-order ----------------
        wqk_sb = w_pool.tile([P, CT, 512], BF16)
        nc.sync.dma_start(wqk_sb[:], wqk.rearrange("(co p) n -> p co n", p=P))

        # single xT tile, one DMA job per 512-wide t-chunk (fewer DIRECT2D
        # programming slots on the sync sequencer)
        xT_t = xt_pool.tile([P, CT, T], BF16, name="xT")
        xT = [xT_t[:, ci, :] for ci in range(CT)]

        def load_xt(tch):
            tsl = slice(tch * 512, (tch + 1) * 512)
            nc.sync.dma_start(
                xT_t[:, :, tsl],
                xt.rearrange("(co p) t -> p co t", p=P)[:, :, tsl])

        load_xt(0)
        wv_sb = w_pool.tile([P, CT, 256], BF16)
        nc.sync.dma_start(wv_sb[:], wv.rearrange("(co p) n -> p co n", p=P))
        for tch in range(1, T // 512):
            load_xt(tch)
        wout_sb = w_pool.tile([P, 2, C], BF16)
        nc.sync.dma_start(wout_sb[:], wout.rearrange("(pr p) n -> p pr n", p=P))

        # ---------------- PE warmup during the DMA ramp ----------------
        pw = ps_qk.tile([P, 512], F32, tag="qk", name="pw")
        for i in range(N_WARMUP):
            nc.tensor.matmul(pw[:], warm_a[:], warm_b[:],
                             start=(i == 0), stop=(i == N_WARMUP - 1))
        # keep the warmup live past DCE with a tiny DRAM write
        wsink = sb_misc.tile([1, 8], F32, tag="wsink", name="wsink")
        nc.vector.tensor_copy(wsink[:], pw[0:1, 0:8])
        dwarm = dram_tmp.tile([1, 8], F32, name="dwarm")
        nc.sync.dma_start(dwarm[:], wsink[:])

        qkt = [qkt_pool.tile([P, T], BF16, tag=f"qkt{ch}", name=f"qkt{ch}")
               for ch in range(4)]
        # V padded to 128 columns per head: a 128-wide bf16 stationary
        # operand gets FWL (fast weight load); PSUM rows 65..127 are unused
        v_sb = [v_pool.tile([P, HEADS_PER_CORE, P], BF16, tag=f"v{ti}",
                            name=f"v{ti}") for ti in range(TT)]

        # ---------------- phase B: Q^T/K^T projections ----------------
        # chunk layout: 0 = pair0 Q (headA|headB), 1 = pair0 K, 2/3 = pair1
        def emit_b(ch, pool, tag, copy_eng):
            for tch in range(T // 512):
                pq = pool.tile([P, 512], F32, tag=tag)
                for ci in range(CT):
                    nc.tensor.matmul(
                        pq[:], wqk_sb[:, ci, ch * P:(ch + 1) * P],
                        xT[ci][:, tch * 512:(tch + 1) * 512],
                        start=(ci == 0), stop=(ci == CT - 1))
                copy_eng(qkt[ch][:, tch * 512:(tch + 1) * 512], pq[:])

        # ACT is idle outside attention; it takes the pair-0 PSUM copies
        emit_b(0, ps_qk, "qk", nc.scalar.copy)
        emit_b(1, ps_qk, "qk", nc.scalar.copy)

        # ---------------- phase C: V (+ones col) ----------------
        for ti in range(TT):
            pv = ps_v.tile([P, 256], F32, tag="v")
            for ci in range(CT):
                nc.tensor.matmul(
                    pv[:], xT[ci][:, ti * P:(ti + 1) * P], wv_sb[:, ci],
                    start=(ci == 0), stop=(ci == CT - 1))
            nc.scalar.copy(
                v_sb[ti][:, :, 0:HD],
                pv[:].rearrange("p (h d) -> p h d", h=HEADS_PER_CORE))
            nc.gpsimd.memset(v_sb[ti][:, :, HD], 1.0)
            nc.gpsimd.memset(v_sb[ti][:, :, HD + 1:], 0.0)

        # close B/C PSUM pools so phase D pools get their banks
        early.close()
        ps_s = ep(tc.tile_pool(name="ps_s", bufs=2, space="PSUM"))
        ps_o = ep(tc.tile_pool(name="ps_o", bufs=1, space="PSUM"))
        ps_by = ep(tc.tile_pool(name="ps_by", bufs=2, space="PSUM"))

        # ---------------- phase D: attention ----------------
        ot = [ot_pool.tile([P, T], BF16, tag=f"ot{p}", name=f"ot{p}")
              for p in range(PAIRS)]

        def emit_out_proj(ti):
            ysb = ysb_pool.tile([P, C], BF16, tag="y", name="ysb")
            for cc in range(C // 512):
                py = ps_by.tile([P, 512], F32, tag="by", name="py")
                for pp in range(PAIRS):
                    nc.tensor.matmul(
                        py[:], ot[pp][:, ti * P:(ti + 1) * P],
                        wout_sb[:, pp, cc * 512:(cc + 1) * 512],
                        start=(pp == 0), stop=(pp == PAIRS - 1))
                # split PSUM->bf16 casts across DVE and ACT
                ysl = slice(cc * 512, (cc + 1) * 512)
                if cc == 0:
                    nc.vector.tensor_copy(ysb[:, ysl], py[:])
                else:
                    nc.scalar.copy(ysb[:, ysl], py[:])
                if ti == TT - 1:
                    # final t-tile: ship each half as soon as it's cast so
                    # the kernel-tail drain starts earlier
                    nc.sync.dma_start(
                        y[ti * P:(ti + 1) * P, ysl], ysb[:, ysl])
            if ti != TT - 1:
                nc.sync.dma_start(y[ti * P:(ti + 1) * P, :], ysb[:])

        pending_norm = []

        def emit_norm(p, qc, po):
            # stash raw O into ot; SBUF->SBUF DMA spreads the denom row
            # into a [32, 32] layout (DVE reciprocal cost is free-dim
            # driven), reciprocal there, DMA back to [2, QC], one sel
            # matmul, multiply ot in place straight from PSUM.
            last = (p == PAIRS - 1 and qc == NQC - 1)
            qsl = slice(qc * QC, (qc + 1) * QC)
            stage = sb_misc.tile([HD + 1, 2 * QC], F32, tag="stage",
                                 name="stage")
            if last:
                # final group: interleave the denom row per 128-q quarter
                # ([h0 | h1] contiguous per quarter); one SBUF->SBUF hop
                # per quarter straight into [2, 128] so the closing
                # out_projs start as early as possible
                den_q = []
                for i in range(4):
                    for half in range(2):
                        nc.vector.tensor_copy(
                            stage[HD:HD + 1,
                                  i * 2 * P + half * P:
                                  i * 2 * P + (half + 1) * P],
                            po[half][HD:HD + 1, i * P:(i + 1) * P])
                    dq = sb_misc.tile([2, P], F32, tag=f"denq{i}",
                                      name=f"denq{i}")
                    nc.sync.dma_start(
                        dq[:], stage[HD:HD + 1, i * 2 * P:(i + 1) * 2 * P])
                    den_q.append(dq)
                for half in range(2):
                    nc.vector.tensor_copy(
                        ot[p][half * HD:(half + 1) * HD, qsl],
                        po[half][0:HD, :])
            else:
                for half in range(2):
                    nc.vector.tensor_copy(
                        stage[HD:HD + 1, half * QC:(half + 1) * QC],
                        po[half][HD:HD + 1, :])
                den32 = sb_misc.tile([32, 32], F32, tag="den", name="den32")
                nc.sync.dma_start(den32[:], stage[HD:HD + 1, :])
                for half in range(2):
                    nc.vector.tensor_copy(
                        ot[p][half * HD:(half + 1) * HD, qsl],
                        po[half][0:HD, :])

            def finish():
                if last:
                    for i in range(4):
                        rq2 = sb_misc.tile([2, P], BF16, tag=f"recipq2{i}",
                                           name=f"recipq2{i}")
                        with nc.allow_low_precision(reason="bf16 recip"):
                            nc.vector.reciprocal(rq2[:], den_q[i][:])
                        pb = ps_by.tile([P, P], F32, tag="by", name="pbq")
                        nc.tensor.matmul(pb[:], sel_sb[:], rq2[:],
                                         start=True, stop=True)
                        isl = slice(qc * QC + i * P, qc * QC + (i + 1) * P)
                        nc.vector.tensor_tensor(
                            ot[p][:, isl], ot[p][:, isl],
                            pb[:], mybir.AluOpType.mult)
                        emit_out_proj(4 * qc + i)
                    return
                recip32 = sb_misc.tile([32, 32], BF16, tag="recip",
                                       name="recip32")
                with nc.allow_low_precision(reason="bf16 recip"):
                    nc.vector.reciprocal(recip32[:], den32[:])
                recip2 = sb_misc.tile([2, QC], BF16, tag="recip2",
                                      name="recip2")
                nc.sync.dma_start(recip2[:], recip32[:])
                pb = ps_by.tile([P, QC], F32, tag="by", name="pb")
                nc.tensor.matmul(pb[:], sel_sb[:], recip2[:],
                                 start=True, stop=True)
                # per-t-tile multiply so out_proj(ti) can start while the
                # later tiles' normalization is still running
                for i in range(4):
                    isl = slice(qc * QC + i * P, qc * QC + (i + 1) * P)
                    nc.vector.tensor_tensor(
                        ot[p][:, isl], ot[p][:, isl],
                        pb[:, i * P:(i + 1) * P], mybir.AluOpType.mult)
                    if p == PAIRS - 1:
                        emit_out_proj(4 * qc + i)
            pending_norm.append(finish)

        def drain_norm():
            while pending_norm:
                pending_norm.pop(0)()

        def emit_d(p):
            qt_t, kt_t = qkt[2 * p], qkt[2 * p + 1]
            for qc in range(NQC):
                kmax = 4 * (qc + 1)
                po = [ps_o.tile([P, QC], F32, tag=f"o{h}", name=f"po{h}")
                      for h in range(2)]
                for kt in range(kmax):
                    r = kt - (kmax - 4)
                    sp = max(r, 0) * P     # valid q-span starts here
                    ksl = slice(kt * P, (kt + 1) * P)
                    qsub = slice(qc * QC + sp, (qc + 1) * QC)
                    ps = ps_s.tile([P, 2 * QC], F32, tag="s")
                    for half in range(2):
                        off = half * QC
                        nc.tensor.matmul(ps[:, off + sp:off + QC],
                                         kt_t[half * HD:(half + 1) * HD, ksl],
                                         qt_t[half * HD:(half + 1) * HD, qsub],
                                         start=True, stop=True,
                                         tile_position=(half * HD, 0))
                    est = est_pool.tile([P, 2 * QC], BF16, tag="est")
                    # one exp instruction per block: the [QC:QC+sp] middle
                    # is never-read garbage, but one wide instruction beats
                    # two narrow ones (~290ns fixed cost each)
                    nc.scalar.activation(est[:, sp:], ps[:, sp:],
                                         mybir.ActivationFunctionType.Exp,
                                         scale=SCALE)
                    if r >= 0:
                        for half in range(2):
                            off = half * QC
                            nc.vector.tensor_tensor(
                                est[:, off + r * P: off + (r + 1) * P],
                                est[:, off + r * P: off + (r + 1) * P],
                                maskr[:], mybir.AluOpType.mult)
                    for half in range(2):
                        nc.tensor.matmul(
                            po[half][:, sp:],
                            v_sb[kt][:, 2 * p + half],
                            est[:, half * QC + sp:(half + 1) * QC],
                            start=(kt == 0), stop=(kt == kmax - 1))
                emit_norm(p, qc, po)
                # deep pipeline in pair 0; drain tighter in pair 1 so only
                # the final group's chain is exposed at the end
                depth = 3 if p == 0 else 2
                while len(pending_norm) >= depth:
                    pending_norm.pop(0)()

        emit_d(0)
        # DVE for pair-1 copies: ACT (exp) paces phase D, keep it clear
        emit_b(2, ps_by, "by", nc.vector.tensor_copy)
        emit_b(3, ps_by, "by", nc.vector.tensor_copy)
        emit_d(1)
        drain_norm()


def _get_nc():
    global _NC_CACHE
    if _NC_CACHE is None:
        _NC_CACHE = _build()
    return _NC_CACHE


def kernel(x, w_qkv, w_out):
    global LAST_RESULTS
    x = np.asarray(x, dtype=np.float32)
    w_qkv = np.asarray(w_qkv, dtype=np.float32)
    w_out = np.asarray(w_out, dtype=np.float32)

    wq, wk, wv = w_qkv[:, 0:C], w_qkv[:, C:2 * C], w_qkv[:, 2 * C:3 * C]

    xt_all = [np.ascontiguousarray(x[b].T).astype(NP_BF16) for b in range(B)]

    in_maps = []
    for c in range(N_CORES):
        b, g = c // 4, c % 4
        heads = [4 * g + i for i in range(HEADS_PER_CORE)]
        cols = lambda w, h: w[:, h * HD:(h + 1) * HD]
        wqk_c = np.concatenate([
            cols(wq, heads[0]), cols(wq, heads[1]),
            cols(wk, heads[0]), cols(wk, heads[1]),
            cols(wq, heads[2]), cols(wq, heads[3]),
            cols(wk, heads[2]), cols(wk, heads[3]),
        ], axis=1)
        wv_c = wv[:, heads[0] * HD:(heads[-1] + 1) * HD]
        wout_c = w_out[heads[0] * HD:(heads[-1] + 1) * HD, :]
        sel_np = np.zeros((2, 128), dtype=NP_BF16)
        sel_np[0, 0:64] = 1.0
        sel_np[1, 64:128] = 1.0
        in_maps.append({
            "xt": xt_all[b],
            "sel": sel_np,
            "wqk": np.ascontiguousarray(wqk_c).astype(NP_BF16),
            "wv": np.ascontiguousarray(wv_c).astype(NP_BF16),
            "wout": np.ascontiguousarray(wout_c).astype(NP_BF16),
        })

    nc = _get_nc()
    res = bass_utils.run_bass_kernel_spmd(
        nc, in_maps, core_ids=list(range(N_CORES)),
        trace=bool(os.environ.get("ATTN_TRACE")))
    LAST_RESULTS = res

    out = np.zeros((B, T, C), dtype=np.float64)
    for c in range(N_CORES):
        out[c // 4] += res.results[c]["y"].astype(np.float64)
    return out.astype(np.float32)
